# revision 1
# baseline (speedup 1.0000x reference)
"""Single-head causal attention (B=4, T=4096, C=1024, H=64) on 8 Trainium2 cores.

Sharding: core c = (batch b=c//2, parity p=c%2). Each core handles the 16
query row-blocks (128 rows) with global block index 2t+p, t=0..15 — parity
interleaving balances causal work exactly across the two cores of a batch,
and makes the SPMD program shape-uniform (key extent for local block t is
256*(t+1), independent of parity; the half-block of waste is masked out).

Device per core (fp16 matmul inputs, fp32 PSUM accumulation):
  k^T,v^T = W^T @ x^T (full 4096 keys), q^T for own 2048 rows.
  For each 128-key block j: S^T[128k, q] = kT_j^T q (scores transposed),
  additive diagonal mask (host-provided, parity-dependent), U = exp(S^T/32)
  via ACT, then [out^T; rowsum] += [v_j | 1]^T U accumulated in PSUM.
Host normalizes (out^T / rowsum), transposes, and scatters rows back.
"""
import numpy as np

B, T, C, H = 4, 4096, 1024, 64
TQ = T // 2              # own query rows per core
NKB = T // 128           # 32 key blocks
NEG = -1e9
N_CORES = 8

_cache = {}


def _build_nc():
    import concourse.bass as bass
    import concourse.tile as tile
    from concourse import bacc, mybir
    from concourse.masks import make_identity

    f32 = mybir.dt.float32
    f16 = mybir.dt.float16
    AF = mybir.ActivationFunctionType

    nc = bacc.Bacc()
    xqT = nc.declare_dram_parameter("xqT", [C, TQ], f16, isOutput=False)
    xkT = nc.declare_dram_parameter("xkT", [C, T], f16, isOutput=False)
    wq = nc.declare_dram_parameter("wq", [C, H], f16, isOutput=False)
    wk = nc.declare_dram_parameter("wk", [C, H], f16, isOutput=False)
    wv = nc.declare_dram_parameter("wv", [C, H], f16, isOutput=False)
    mask_e = nc.declare_dram_parameter("mask_e", [128, 128], f32, isOutput=False)
    mask_o = nc.declare_dram_parameter("mask_o", [128, 128], f32, isOutput=False)
    oacc = nc.declare_dram_parameter("oacc", [H + 1, TQ], f32, isOutput=True)

    CA = C // 128  # 8 contraction chunks

    with tile.TileContext(nc) as tc:
        with tc.tile_pool(name="consts", bufs=1) as consts, \
             tc.tile_pool(name="big", bufs=1) as big:
            wq_sb = consts.tile([128, CA, H], f16, tag="wq")
            wk_sb = consts.tile([128, CA, H], f16, tag="wk")
            wv_sb = consts.tile([128, CA, H], f16, tag="wv")
            me_sb = consts.tile([128, 128], f32, tag="me")
            mo_sb = consts.tile([128, 128], f32, tag="mo")
            ident = consts.tile([64, 64], f16, tag="ident")
            for w_sb, w_dr in ((wq_sb, wq), (wk_sb, wk), (wv_sb, wv)):
                nc.sync.dma_start(out=w_sb, in_=w_dr.rearrange("(a p) h -> p a h", p=128))
            nc.sync.dma_start(out=me_sb, in_=mask_e[:, :])
            nc.sync.dma_start(out=mo_sb, in_=mask_o[:, :])
            make_identity(nc, ident)

            kT_sb = big.tile([64, T], f16, tag="kT")
            qT_sb = big.tile([64, TQ], f16, tag="qT")
            v_all = big.tile([128, NKB, H + 1], f16, tag="v")
            nc.vector.memset(v_all[:, :, H:H + 1], 1.0)

            xkT_r = xkT.rearrange("(a p) t -> p a t", p=128)
            xqT_r = xqT.rearrange("(a p) t -> p a t", p=128)

            # ---- Phase B: k^T, v^T projections + v transpose; q^T ----
            with tc.tile_pool(name="xt", bufs=3) as xtp, \
                 tc.tile_pool(name="vt16", bufs=2) as vt16p, \
                 tc.tile_pool(name="pproj", bufs=4, space="PSUM") as pp, \
                 tc.tile_pool(name="pvt", bufs=2, space="PSUM") as pvt:
                for g in range(T // 512):
                    xk_t = xtp.tile([128, CA, 512], f16, tag="xt")
                    nc.sync.dma_start(out=xk_t, in_=xkT_r[:, :, 512 * g:512 * (g + 1)])
                    ps_k = pp.tile([64, 512], f32, tag="pp")
                    ps_v = pp.tile([64, 512], f32, tag="pp")
                    for a in range(CA):
                        nc.tensor.matmul(ps_k, lhsT=wk_sb[:, a, :], rhs=xk_t[:, a, :],
                                         start=(a == 0), stop=(a == CA - 1))
                    for a in range(CA):
                        nc.tensor.matmul(ps_v, lhsT=wv_sb[:, a, :], rhs=xk_t[:, a, :],
                                         start=(a == 0), stop=(a == CA - 1))
                    nc.scalar.activation(kT_sb[:, 512 * g:512 * (g + 1)], ps_k, AF.Copy)
                    vt16 = vt16p.tile([64, 512], f16, tag="vt16")
                    nc.scalar.activation(vt16, ps_v, AF.Copy)
                    for u in range(4):
                        j = 4 * g + u
                        ps_t = pvt.tile([128, 64], f16, tag="pvt")
                        nc.tensor.transpose(ps_t, vt16[:, 128 * u:128 * (u + 1)], ident)
                        nc.vector.tensor_copy(v_all[:, j, 0:H], ps_t)
                for g in range(TQ // 512):
                    xq_t = xtp.tile([128, CA, 512], f16, tag="xt")
                    nc.sync.dma_start(out=xq_t, in_=xqT_r[:, :, 512 * g:512 * (g + 1)])
                    ps_q = pp.tile([64, 512], f32, tag="pp")
                    for a in range(CA):
                        nc.tensor.matmul(ps_q, lhsT=wq_sb[:, a, :], rhs=xq_t[:, a, :],
                                         start=(a == 0), stop=(a == CA - 1))
                    nc.scalar.activation(qT_sb[:, 512 * g:512 * (g + 1)], ps_q, AF.Copy)

            # ---- Phase C: scores^T -> mask -> exp -> AV accumulate ----
            with tc.tile_pool(name="pss", bufs=3, space="PSUM") as pss, \
                 tc.tile_pool(name="po", bufs=1, space="PSUM") as pop, \
                 tc.tile_pool(name="u16", bufs=4) as up:
                ps_o = pop.tile([H + 1, TQ], f32, tag="po")
                for j in range(NKB):
                    t0 = j // 2
                    q_start = 128 * t0
                    ncols = TQ - q_start
                    first = min(ncols, 512 - (q_start % 512))
                    sizes = [first] + [512] * ((ncols - first) // 512)
                    cs = q_start
                    for ci, cn in enumerate(sizes):
                        ps_s = pss.tile([128, 512], f32, tag="pss")
                        s_ap = ps_s[:, 0:cn]
                        nc.tensor.matmul(s_ap, lhsT=kT_sb[:, 128 * j:128 * (j + 1)],
                                         rhs=qT_sb[:, cs:cs + cn], start=True, stop=True)
                        if ci == 0:
                            m = me_sb if j % 2 == 0 else mo_sb
                            nc.vector.tensor_add(ps_s[:, 0:128], ps_s[:, 0:128], m)
                        u_sb = up.tile([128, 512], f16, tag="u16")
                        nc.scalar.activation(u_sb[:, 0:cn], s_ap, AF.Exp,
                                             scale=float(C ** -0.5))
                        nc.tensor.matmul(ps_o[:, cs:cs + cn], lhsT=v_all[:, j, :],
                                         rhs=u_sb[:, 0:cn], start=(j == 0),
                                         stop=(j == NKB - 1), skip_group_check=True)
                        cs += cn

                osb = big.tile([H + 1, TQ], f32, tag="osb")
                nc.vector.tensor_copy(osb, ps_o)
                nc.sync.dma_start(out=oacc[:, :], in_=osb)
    nc.compile()
    return nc


def _get_nc():
    if "nc" not in _cache:
        _cache["nc"] = _build_nc()
    return _cache["nc"]


def _core_masks(p):
    kk = np.arange(128)[:, None]
    i = np.arange(128)[None, :]
    tri = np.where(i >= kk, 0.0, NEG).astype(np.float32)
    if p == 0:
        return tri, np.full((128, 128), NEG, np.float32)
    return np.zeros((128, 128), np.float32), tri


def make_in_maps(x, Wk, Wq, Wv):
    wk16 = np.ascontiguousarray(Wk).astype(np.float16)
    wq16 = np.ascontiguousarray(Wq).astype(np.float16)
    wv16 = np.ascontiguousarray(Wv).astype(np.float16)
    in_maps = []
    for c in range(N_CORES):
        b, p = c // 2, c % 2
        xb = np.asarray(x[b])
        xq = xb.reshape(NKB, 128, C)[p::2].reshape(TQ, C)
        xqT = np.ascontiguousarray(xq.T.astype(np.float16))
        xkT = np.ascontiguousarray(xb.T.astype(np.float16))
        me, mo = _core_masks(p)
        in_maps.append({"xqT": xqT, "xkT": xkT, "wq": wq16, "wk": wk16,
                        "wv": wv16, "mask_e": me, "mask_o": mo})
    return in_maps


def postprocess(results):
    out = np.zeros((B, T, H), np.float32)
    for c in range(N_CORES):
        b, p = c // 2, c % 2
        acc = results[c]["oacc"]
        o = (acc[0:H] / acc[H:H + 1]).T
        out[b].reshape(NKB, 128, H)[p::2] = o.reshape(16, 128, H)
    return out


def run_full(x, Wk, Wq, Wv, trace=False):
    from concourse.bass_utils import run_bass_kernel_spmd
    nc = _get_nc()
    in_maps = make_in_maps(x, Wk, Wq, Wv)
    res = run_bass_kernel_spmd(nc, in_maps, list(range(N_CORES)), trace=trace)
    return postprocess(res.results), res


def kernel(x, Wk, Wq, Wv):
    out, _ = run_full(x, Wk, Wq, Wv)
    return out
